# revision 1
# baseline (speedup 1.0000x reference)
"""ChebConv-with-spatial-attention Trainium2 kernel.

out[t,b,m,o] = relu( sum_{k,n,f} cheb[k,n,m] * s_a[b,n,m] * X[b,n,f,t] * Theta[k,f,o] )

Shapes: B=16, N=512, F=32, T=24, K=3, O=64.  All fp32 in/out.

Strategy (8 NeuronCores, data-parallel over batch, 2 batches per core):
  stage 0 (DVE):  A_kb[n,m] = cheb_k[n,m] * s_a_b[n,m]            (SBUF, elementwise)
  stage 1 (PE):   Y_k[(tj,f), m] = sum_n X[b,n,tg*4+tj,f] * A_kb[n,m]
                  - lhsT = X block [128n, 128(tj,f)], rhs = A [128n, 512m]
                  - accumulate over 4 n-tiles into PSUM [128, 3*512] (one 512-col bank per k)
  stage 2 (PE):   out[o, m] (b, t) = sum_{k,f} Theta[k,f,o] * Y_k[(tj,f), m]
                  - lhsT = Theta strip [32f, 64o] at row 32*tj, rhs = Y strip [32, 512]
                  - k accumulated in PSUM; 4 t's packed in one [128, 1024] PSUM tile
  relu on evac, DMA out as [T, BL, O, N]; host transposes to [T,B,N,O].

Matmul operands run as bf16 (MM_MODE="bf16") or fp32 via the full-rate
float32r PE mode (MM_MODE="fp32r").
"""

import sys

sys.path.insert(0, "/opt/trn_rl_repo")

import numpy as np
import ml_dtypes

import concourse.bacc as bacc
import concourse.tile as tile
from concourse import mybir
from concourse.bass_utils import run_bass_kernel_spmd

B, N, F, T, K, O = 16, 512, 32, 24, 3, 64
NC = 8
BL = B // NC          # batches per core = 2
NT = N // 128         # n tiles = 4
TG = T // 4           # t-groups of 4 = 6
FT = F * T            # 768

MM_MODE = "bf16"      # "bf16" | "fp32r"


def _build_program(mode):
    io_dt = mybir.dt.bfloat16 if mode == "bf16" else mybir.dt.float32
    nc = bacc.Bacc("TRN2", target_bir_lowering=False, debug=False, num_devices=NC)

    # X pre-transposed on host to [BL, N, T, F] so a [128, 128] slice of the
    # free dim covers 4 consecutive t's of all 32 f's.
    X_d = nc.dram_tensor("X", [BL, N, T * F], io_dt, kind="ExternalInput").ap()
    SA_d = nc.dram_tensor("SA", [BL, N, N], io_dt, kind="ExternalInput").ap()
    CH_d = nc.dram_tensor("CH", [K, N, N], io_dt, kind="ExternalInput").ap()
    # Theta pre-tiled on host to [K, 128, O] (partition = tj*32+f).
    TH_d = nc.dram_tensor("TH", [K, 128, O], io_dt, kind="ExternalInput").ap()
    OUT_d = nc.dram_tensor("OUT", [T, BL, O, N], mybir.dt.float32, kind="ExternalOutput").ap()

    def mm(ap):
        return ap.bitcast(mybir.dt.float32r) if mode == "fp32r" else ap

    with tile.TileContext(nc) as tc:
        with (
            tc.tile_pool(name="const", bufs=1) as cpool,
            tc.tile_pool(name="ypsum", bufs=2, space="PSUM") as ypool,
            tc.tile_pool(name="opsum", bufs=1, space="PSUM") as opool,
            tc.tile_pool(name="ysb", bufs=2) as ysbpool,
            tc.tile_pool(name="osb", bufs=2) as osbpool,
        ):
            xsb = cpool.tile([128, BL * NT * FT], io_dt, tag="xsb")
            chsb = cpool.tile([128, K * NT * N], io_dt, tag="chsb")
            sasb = cpool.tile([128, BL * NT * N], io_dt, tag="sasb")
            asb = cpool.tile([128, K * BL * NT * N], io_dt, tag="asb")
            thsb = cpool.tile([128, K * O], io_dt, tag="thsb")

            def xoff(b, n4):
                return (b * NT + n4) * FT

            def choff(k, n4):
                return (k * NT + n4) * N

            def saoff(b, n4):
                return (b * NT + n4) * N

            def aoff(k, b, n4):
                return ((k * BL + b) * NT + n4) * N

            # ---- input DMAs (n4-major so the pipeline can start early) ----
            # alternate between the two HWDGE rings (SP / ACT) so loads
            # don't serialize on one ring
            _ring = [nc.sync, nc.scalar]
            _rr = [0]

            def load(dst, src):
                _ring[_rr[0] % 2].dma_start(dst, src)
                _rr[0] += 1

            for k in range(K):
                load(thsb[:, k * O:(k + 1) * O], TH_d[k])
            # b=0's working set first: the first stage-1 group needs all four
            # n-tiles of X[0], cheb, and A[.,0,.] before it can finish
            for b in range(BL):
                for n4 in range(NT):
                    load(
                        xsb[:, xoff(b, n4):xoff(b, n4) + FT],
                        X_d[b, n4 * 128:(n4 + 1) * 128, :],
                    )
                    if b == 0:
                        for k in range(K):
                            load(
                                chsb[:, choff(k, n4):choff(k, n4) + N],
                                CH_d[k, n4 * 128:(n4 + 1) * 128, :],
                            )
                    load(
                        sasb[:, saoff(b, n4):saoff(b, n4) + N],
                        SA_d[b, n4 * 128:(n4 + 1) * 128, :],
                    )

            # ---- stage 0: A = cheb * s_a (DVE) ----
            for b in range(BL):
                for n4 in range(NT):
                    for k in range(K):
                        nc.vector.tensor_mul(
                            asb[:, aoff(k, b, n4):aoff(k, b, n4) + N],
                            chsb[:, choff(k, n4):choff(k, n4) + N],
                            sasb[:, saoff(b, n4):saoff(b, n4) + N],
                        )

            groups = [(b, tg) for b in range(BL) for tg in range(TG)]

            def stage1(b, tg):
                yp = ypool.tile([128, K * N], mybir.dt.float32, tag="yp")
                for n4 in range(NT):
                    xw = xsb[:, xoff(b, n4) + tg * 128: xoff(b, n4) + (tg + 1) * 128]
                    for k in range(K):
                        nc.tensor.matmul(
                            yp[:, k * N:(k + 1) * N],
                            mm(xw),
                            mm(asb[:, aoff(k, b, n4):aoff(k, b, n4) + N]),
                            start=(n4 == 0),
                            stop=(n4 == NT - 1),
                        )
                ysb = ysbpool.tile([128, K * N], io_dt, tag="ysb")
                nc.vector.tensor_copy(ysb[:, 0:1024], yp[:, 0:1024])
                nc.scalar.copy(ysb[:, 1024:1536], yp[:, 1024:1536])
                return ysb

            def stage2(b, tg, ysb):
                op = opool.tile([128, 1024], mybir.dt.float32, tag="op")
                # Two phases of two concurrent k-accumulation chains. A PSUM
                # accumulation group owns its whole bank, so concurrent chains
                # must target different banks: (tj0: bank0/part0-63/rows0-31,
                # tj3: bank1/part64-127/rows96-127), then (tj1, tj2).
                for ta, tb in ((0, 3), (1, 2)):
                    for k in range(K):
                        for tj in (ta, tb):
                            pj, c = tj % 2, tj // 2
                            nc.tensor.matmul(
                                op[64 * pj:64 * pj + 64, 512 * c:512 * c + 512],
                                mm(thsb[32 * tj:32 * tj + 32, k * O:(k + 1) * O]),
                                mm(ysb[32 * tj:32 * tj + 32, k * N:(k + 1) * N]),
                                start=(k == 0),
                                stop=(k == K - 1),
                                tile_position=(32 * tj, 64 * pj),
                            )
                ob = osbpool.tile([128, 1024], mybir.dt.float32, tag="ob")
                nc.scalar.activation(ob[:], op[:], mybir.ActivationFunctionType.Relu)
                # t = tg*4 + 2*c + pj ; SBUF [64, 512] quadrant -> DRAM
                # [64, 512], alternating between the two HWDGE rings
                for c in range(2):
                    for pj in range(2):
                        t = tg * 4 + 2 * c + pj
                        _ring[_rr[0] % 2].dma_start(
                            OUT_d[t, b],
                            ob[64 * pj:64 * pj + 64, 512 * c:512 * c + 512],
                        )
                        _rr[0] += 1

            # software-pipeline: stage2(g-1) is emitted after stage1(g) so the
            # PE never waits on the PSUM->SBUF evacuation of the current group
            prev = None
            for g, (b, tg) in enumerate(groups):
                ysb = stage1(b, tg)
                if prev is not None:
                    stage2(*prev)
                prev = (b, tg, ysb)
            stage2(*prev)

    nc.compile()
    return nc


_prog_cache = {}


def _get_program(mode):
    if mode not in _prog_cache:
        _prog_cache[mode] = _build_program(mode)
    return _prog_cache[mode]


def kernel(X, s_a, cheb, Theta):
    np_dt = ml_dtypes.bfloat16 if MM_MODE == "bf16" else np.float32
    # host-side prep
    Xh = np.ascontiguousarray(X.transpose(0, 1, 3, 2)).reshape(B, N, T * F).astype(np_dt)
    sah = np.ascontiguousarray(s_a).astype(np_dt)
    chh = np.ascontiguousarray(cheb).astype(np_dt)
    thh = np.tile(Theta, (1, 4, 1)).astype(np_dt)          # [K, 128, O]

    in_maps = []
    for c in range(NC):
        lo, hi = c * BL, (c + 1) * BL
        in_maps.append({
            "X": Xh[lo:hi],
            "SA": sah[lo:hi],
            "CH": chh,
            "TH": thh,
        })

    nc = _get_program(MM_MODE)
    res = run_bass_kernel_spmd(nc, in_maps, list(range(NC)))
    # per-core OUT: [T, BL, O, N] -> full [T, B, N, O]
    out = np.concatenate([r["OUT"] for r in res.results], axis=1)
    return np.ascontiguousarray(out.transpose(0, 1, 3, 2))

